# revision 29
# baseline (speedup 1.0000x reference)
"""LocallyConnected2d Trainium2 kernel (bf16).

y[b,o,h,w] = sum_{i,ky,kx} x[b,i,h+ky-1,w+kx-1] * weight[i,o,h,w,ky,kx] + bias[o,h,w]

Shapes: x [64,64,32,32], weight [64,64,32,32,3,3], bias [64,32,32] -> y [64,64,32,32].

Strategy
--------
Spatial sharding over H_out: 8 cores x 4 output rows each. All matmul inputs
are bf16 (tolerance is 2e-2; bf16 keeps rel err ~3e-3). Output is written
bf16; bias is added on host during unpack (exact fp32 add, no device cost).

Per output location (h,w): K=576 x M=64(cout) x N=64(batch), split into
6 PSUM-accumulating matmuls:
  j=0,1,2  K=128 chunks pairing offsets (ky=j,kx=0)+(ky=j,kx=1); the bottom
           64 partitions read the x slab shifted by 1 column (host-packed
           second copy), so one rhs AP serves both offsets.
  g=0,1,2  singles (0,2),(1,2),(2,2): K=64 matmuls on partitions 0-63
           reading the unshifted slab at rc3 + 34*g.

Schedule (what the ~2.5x over the fp32 baseline came from, per perfetto):
- Eight consecutive locations share ONE psum bank as a single accumulation
  group (start=True marks the whole 2KB bank pending-zero; later matmuls
  zero-then-accumulate their own slots), so the PSUM drain is one
  512-element copy per 8 locations and the PE never stalls on banks.
- Within a group all 24 K=128 fulls run before the 24 K=64 singles: a
  128-row <-> 64-row LDWEIGHTS transition costs a ~180ns pipeline drain,
  so it is paid twice per group instead of twice per location.
- ALL inputs ride one HWDGE ring in exact consumption order (two rings
  share the 16 SDMA engines round-robin, so splitting inputs reorders
  arrivals vs need and starves the PE); outputs ride the other ring.
  x is split into per-row tiles interleaved with per-h weight chunks.
- A short junk-matmul burst bridges the initial DMA fill so the HAM
  clock-gate is warm (2.4 GHz, ~32ns/MM vs 53 cold) when real work starts.
The kernel is DMA-bound end to end: ~12.8 MB of input at the ~270 GB/s
per-core share of HBM (8 cores active) sets the ~55us floor the matmul
stream trails by one weight chunk.
"""

import sys

sys.path.insert(0, "/opt/trn_rl_repo")

import numpy as np
import ml_dtypes

BF16 = ml_dtypes.bfloat16

B, CIN, COUT, H, W = 64, 64, 64, 32, 32
K = 3
HOUT, WOUT = 32, 32
NCORES = 8
ROWS = HOUT // NCORES  # output rows per core
SLAB_R = ROWS + 2      # x rows needed per core (halo)
SLAB_C = W + 2         # padded width
RC = SLAB_R * SLAB_C   # flattened (row, col) length
RC_LO = 3 * SLAB_C     # x rows 0-2 in the lo tile, 3-5 in the hi tile
PW = 8                 # output locations per psum bank / drain group

# full chunk pairing: j=0..2 -> (j,0)+(j,1); singles (0,2),(1,2),(2,2)
PAIRS = [((0, 0), (0, 1)), ((1, 0), (1, 1)), ((2, 0), (2, 1))]
SINGLES = [(0, 2), (1, 2), (2, 2)]

N_WARM = 18        # junk matmuls to warm the PE clock gate during DMA fill
WARM_N = 512       # their moving free dim

_nc_cache = {}


def _build_bass():
    import concourse.bass as bass
    import concourse.tile as tile
    from concourse import bacc, mybir

    f32 = mybir.dt.float32
    bf16 = mybir.dt.bfloat16
    nc = bacc.Bacc(None, target_bir_lowering=False)

    xr_d = [
        nc.dram_tensor(f"xr{r}", (128, SLAB_C, B), bf16, kind="ExternalInput")
        for r in range(SLAB_R)
    ]
    wmain_d = nc.dram_tensor("wmain", (ROWS, 128, WOUT, 3, COUT), bf16, kind="ExternalInput")
    wtail_d = nc.dram_tensor("wtail", (ROWS, 64, WOUT, 3, COUT), bf16, kind="ExternalInput")
    out_d = nc.dram_tensor("out", (ROWS, COUT, WOUT, B), bf16, kind="ExternalOutput")

    with tile.TileContext(nc) as tc:
        with (
            tc.tile_pool(name="xpool", bufs=1) as xpool,
            tc.tile_pool(name="wpool", bufs=4) as wpool,
            tc.tile_pool(name="opool", bufs=2) as opool,
            tc.tile_pool(name="spool", bufs=1) as spool,
            tc.tile_pool(name="psum", bufs=7, space=bass.MemorySpace.PSUM) as psum,
            tc.tile_pool(name="psumw", bufs=1, space=bass.MemorySpace.PSUM) as psumw,
        ):
            # PE warm-up: junk matmuls keep the HAM activity monitor busy
            # while the inputs stream in, so the real stream starts (and
            # stays) at 2.4 GHz -- warm pace is ~38ns/MM vs 64 cold.
            scratch = spool.tile([128, WARM_N], bf16, tag="scratch")
            nc.vector.memset(scratch[:], 0)
            wps = psumw.tile([64, WARM_N], f32, tag="warm")
            for i in range(N_WARM):
                nc.tensor.matmul(
                    wps[:], scratch[:, 0:64], scratch[:],
                    start=True, stop=True,
                )

            xrs = []
            for r in range(SLAB_R):
                xr = xpool.tile([128, SLAB_C, B], bf16, tag=f"xr{r}")
                xrs.append(xr)

            def dma_next(dst, src):
                nc.sync.dma_start(dst, src)

            def xload(r):
                dma_next(xrs[r][:], xr_d[r][:])

            def xap(rc, lo64=False):
                r, c = divmod(rc, SLAB_C)
                t = xrs[r]
                return t[0:64, c, :] if lo64 else t[:, c, :]

            # critical prefix first: the first psum group's j=0 matmuls
            # need only wm[h0][w0:8] + x row 0 (~1 MB)
            W8 = 8
            wms, wts = [], []
            for h in range(ROWS):
                wm = wpool.tile([128, WOUT, 3, COUT], bf16, tag="wm")
                wt = wpool.tile([64, WOUT, 3, COUT], bf16, tag="wt")
                wms.append(wm)
                wts.append(wt)
            # ALL inputs on one ring in exact consumption order: two rings
            # share the 16 SDMA engines round-robin, so splitting inputs
            # reorders arrivals vs need and starves the PE mid-stream.
            # Outputs and shift-copies ride the other ring.
            dma_next(wms[0][:, 0:W8], wmain_d[0][:, 0:W8])
            dma_next(wts[0][:, 0:W8], wtail_d[0][:, 0:W8])
            xload(0)
            xload(1)
            xload(2)
            dma_next(wms[0][:, W8:WOUT], wmain_d[0][:, W8:WOUT])
            dma_next(wts[0][:, W8:WOUT], wtail_d[0][:, W8:WOUT])
            xload(3)
            for h in range(1, ROWS):
                dma_next(wms[h][:], wmain_d[h])
                dma_next(wts[h][:], wtail_d[h])
                if h + 3 < SLAB_R:
                    xload(h + 3)

            for h in range(ROWS):
                wm, wt = wms[h], wts[h]
                ot = opool.tile([COUT, WOUT, B], bf16, tag="out")

                for w0 in range(0, WOUT, PW):
                    ps = psum.tile([COUT, PW, B], f32, tag="ps")
                    # all K=128 fulls first (j-major: j=0 only needs x row
                    # h+0, so compute starts before later rows land), then
                    # all K=64 singles: the 128-row <-> 64-row LDWEIGHTS
                    # transition stalls the PE for a pipeline drain
                    # (~180ns), so pay it twice per group, not per location
                    for j in range(3):
                        for dw in range(PW):
                            w = w0 + dw
                            rc = (h + j) * SLAB_C + w
                            nc.tensor.matmul(
                                ps[:, dw, :],
                                wm[:, w, j, :],
                                xap(rc),
                                start=(dw == 0 and j == 0),
                                stop=False,
                            )
                    for dw in range(PW):
                        w = w0 + dw
                        for g in range(3):
                            rc = (h + g) * SLAB_C + (w + 2)
                            nc.tensor.matmul(
                                ps[:, dw, :],
                                wt[:, w, g, :],
                                xap(rc, lo64=True),
                                start=False,
                                stop=(dw == PW - 1 and g == 2),
                            )
                    nc.any.tensor_copy(ot[:, w0 : w0 + PW, :], ps[:])
                    nc.scalar.dma_start(
                        out_d[h][:, w0 : w0 + PW], ot[:, w0 : w0 + PW, :]
                    )

    nc.compile()
    return nc


def get_nc():
    if "nc" not in _nc_cache:
        _nc_cache["nc"] = _build_bass()
    return _nc_cache["nc"]


def _shift(s, d):
    """s: [64, RC, B]; returns s advanced by d blocks along axis 1, zero-filled."""
    out = np.zeros_like(s)
    out[:, : RC - d, :] = s[:, d:, :]
    return out


def pack_inputs(x, weight, bias):
    """Returns list of per-core in_maps (numpy, C-contiguous)."""
    x = np.asarray(x, dtype=np.float32)
    weight = np.asarray(weight, dtype=np.float32)

    # padded x: [B, CIN, H+2, W+2]
    xp = np.zeros((B, CIN, H + 2, W + 2), dtype=np.float32)
    xp[:, :, 1:-1, 1:-1] = x

    # weight -> [h, w, ky, kx, cin, cout]
    wt = np.ascontiguousarray(np.transpose(weight, (2, 3, 4, 5, 0, 1)))

    ky0s = np.array([p[0][0] for p in PAIRS])
    kx0s = np.array([p[0][1] for p in PAIRS])
    ky1s = np.array([p[1][0] for p in PAIRS])
    kx1s = np.array([p[1][1] for p in PAIRS])
    kys = np.array([s[0] for s in SINGLES])
    kxs = np.array([s[1] for s in SINGLES])

    in_maps = []
    for c in range(NCORES):
        h0 = c * ROWS
        # x slab rows h0-1 .. h0+ROWS (SLAB_R rows of padded x)
        slab = xp[:, :, h0 : h0 + SLAB_R, :]  # [B, CIN, SLAB_R, SLAB_C]
        s = np.transpose(slab, (1, 2, 3, 0)).reshape(CIN, RC, B)  # [cin, rc, b]

        wh = wt[h0 : h0 + ROWS]  # [ROWS, w, ky, kx, cin, cout]
        top = wh[:, :, ky0s, kx0s]  # [ROWS, w, j, cin, cout]
        bot = wh[:, :, ky1s, kx1s]
        # -> [ROWS, cin, w, j, cout]
        top = np.transpose(top, (0, 3, 1, 2, 4))
        bot = np.transpose(bot, (0, 3, 1, 2, 4))
        wmain = np.concatenate([top, bot], axis=1)  # [ROWS, 128, w, 3, cout]
        wtail = np.transpose(wh[:, :, kys, kxs], (0, 3, 1, 2, 4))  # [ROWS, cin, w, 3, cout]

        xab = np.concatenate([s, _shift(s, 1)], axis=0).astype(BF16)  # [128, RC, B]
        m = {
            "wmain": np.ascontiguousarray(wmain.astype(BF16)),
            "wtail": np.ascontiguousarray(wtail.astype(BF16)),
        }
        for r in range(SLAB_R):
            m[f"xr{r}"] = np.ascontiguousarray(
                xab[:, r * SLAB_C : (r + 1) * SLAB_C]
            )
        in_maps.append(m)
    return in_maps


def unpack_outputs(results, bias):
    """results: per-core out_maps with 'out' [ROWS, COUT, WOUT, B] bf16."""
    full = np.concatenate(
        [np.asarray(r["out"]).astype(np.float32) for r in results], axis=0
    )
    # [HOUT, COUT, WOUT, B] -> [B, COUT, HOUT, WOUT]
    y = np.ascontiguousarray(np.transpose(full, (3, 1, 0, 2)))
    y += np.asarray(bias, dtype=np.float32)[None]
    return y


def run(in_maps, **kwargs):
    from concourse import bass_utils

    nc = get_nc()
    return bass_utils.run_bass_kernel_spmd(
        nc, in_maps, core_ids=list(range(NCORES)), **kwargs
    )


def kernel(x, weight, bias):
    in_maps = pack_inputs(x, weight, bias)
    res = run(in_maps)
    return unpack_outputs(res.results, bias)


if __name__ == "__main__":
    rng = np.random.default_rng(0)
    x = rng.standard_normal((B, CIN, H, W), dtype=np.float32)
    weight = rng.standard_normal((CIN, COUT, HOUT, WOUT, K, K), dtype=np.float32)
    bias = rng.standard_normal((COUT, HOUT, WOUT), dtype=np.float32)
    y = kernel(x, weight, bias)
    print("out", y.shape, y.dtype)


# revision 30
# speedup vs baseline: 1.0748x; 1.0748x over previous
"""LocallyConnected2d Trainium2 kernel (bf16).

y[b,o,h,w] = sum_{i,ky,kx} x[b,i,h+ky-1,w+kx-1] * weight[i,o,h,w,ky,kx] + bias[o,h,w]

Shapes: x [64,64,32,32], weight [64,64,32,32,3,3], bias [64,32,32] -> y [64,64,32,32].

Strategy
--------
Spatial sharding over H_out: 8 cores x 4 output rows each. All matmul inputs
are bf16 (tolerance is 2e-2; bf16 keeps rel err ~3e-3). Output is written
bf16; bias is added on host during unpack (exact fp32 add, no device cost).

Per output location (h,w): K=576 x M=64(cout) x N=64(batch), split into
6 PSUM-accumulating matmuls:
  j=0,1,2  K=128 chunks pairing offsets (ky=j,kx=0)+(ky=j,kx=1); the bottom
           64 partitions read the x slab shifted by 1 column (host-packed
           second copy), so one rhs AP serves both offsets.
  g=0,1,2  singles (0,2),(1,2),(2,2): K=64 matmuls on partitions 0-63
           reading the unshifted slab at rc3 + 34*g.

Schedule (what the ~2.5x over the fp32 baseline came from, per perfetto):
- Eight consecutive locations share ONE psum bank as a single accumulation
  group (start=True marks the whole 2KB bank pending-zero; later matmuls
  zero-then-accumulate their own slots), so the PSUM drain is one
  512-element copy per 8 locations and the PE never stalls on banks.
- Within a group all 24 K=128 fulls run before the 24 K=64 singles: a
  128-row <-> 64-row LDWEIGHTS transition costs a ~180ns pipeline drain,
  so it is paid twice per group instead of twice per location.
- ALL inputs ride one HWDGE ring in exact consumption order (two rings
  share the 16 SDMA engines round-robin, so splitting inputs reorders
  arrivals vs need and starves the PE); outputs ride the other ring.
  x is split into per-row tiles interleaved with per-h weight chunks.
- A short junk-matmul burst bridges the initial DMA fill so the HAM
  clock-gate is warm (2.4 GHz, ~32ns/MM vs 53 cold) when real work starts.
The kernel is DMA-bound end to end: ~12.8 MB of input at the ~270 GB/s
per-core share of HBM (8 cores active) sets the ~55us floor the matmul
stream trails by one weight chunk.
"""

import sys

sys.path.insert(0, "/opt/trn_rl_repo")

import numpy as np
import ml_dtypes

BF16 = ml_dtypes.bfloat16

B, CIN, COUT, H, W = 64, 64, 64, 32, 32
K = 3
HOUT, WOUT = 32, 32
NCORES = 8
ROWS = HOUT // NCORES  # output rows per core
SLAB_R = ROWS + 2      # x rows needed per core (halo)
SLAB_C = W + 2         # padded width
RC = SLAB_R * SLAB_C   # flattened (row, col) length
RC_LO = 3 * SLAB_C     # x rows 0-2 in the lo tile, 3-5 in the hi tile
PW = 8                 # output locations per psum bank / drain group

# full chunk pairing: j=0..2 -> (j,0)+(j,1); singles (0,2),(1,2),(2,2)
PAIRS = [((0, 0), (0, 1)), ((1, 0), (1, 1)), ((2, 0), (2, 1))]
SINGLES = [(0, 2), (1, 2), (2, 2)]

N_WARM = 18        # junk matmuls to warm the PE clock gate during DMA fill
WARM_N = 512       # their moving free dim

_nc_cache = {}


def _build_bass():
    import concourse.bass as bass
    import concourse.tile as tile
    from concourse import bacc, mybir

    f32 = mybir.dt.float32
    bf16 = mybir.dt.bfloat16
    nc = bacc.Bacc(None, target_bir_lowering=False)

    xr_d = [
        nc.dram_tensor(f"xr{r}", (128, SLAB_C, B), bf16, kind="ExternalInput")
        for r in range(SLAB_R)
    ]
    wmain_d = nc.dram_tensor("wmain", (ROWS, 128, WOUT, 3, COUT), bf16, kind="ExternalInput")
    wtail_d = nc.dram_tensor("wtail", (ROWS, 64, WOUT, 3, COUT), bf16, kind="ExternalInput")
    out_d = nc.dram_tensor("out", (ROWS, COUT, WOUT, B), bf16, kind="ExternalOutput")

    with tile.TileContext(nc) as tc:
        with (
            tc.tile_pool(name="xpool", bufs=1) as xpool,
            tc.tile_pool(name="wpool", bufs=4) as wpool,
            tc.tile_pool(name="opool", bufs=2) as opool,
            tc.tile_pool(name="spool", bufs=1) as spool,
            tc.tile_pool(name="psum", bufs=7, space=bass.MemorySpace.PSUM) as psum,
            tc.tile_pool(name="psumw", bufs=1, space=bass.MemorySpace.PSUM) as psumw,
        ):
            # PE warm-up: junk matmuls keep the HAM activity monitor busy
            # while the inputs stream in, so the real stream starts (and
            # stays) at 2.4 GHz -- warm pace is ~38ns/MM vs 64 cold.
            scratch = spool.tile([128, WARM_N], bf16, tag="scratch")
            nc.vector.memset(scratch[:], 0)
            wps = psumw.tile([64, WARM_N], f32, tag="warm")
            for i in range(N_WARM):
                nc.tensor.matmul(
                    wps[:], scratch[:, 0:64], scratch[:],
                    start=True, stop=True,
                )

            xrs = []
            for r in range(SLAB_R):
                xr = xpool.tile([128, SLAB_C, B], bf16, tag=f"xr{r}")
                xrs.append(xr)

            def dma_next(dst, src):
                nc.sync.dma_start(dst, src)

            def xload(r):
                dma_next(xrs[r][:], xr_d[r][:])

            def xap(rc, lo64=False):
                r, c = divmod(rc, SLAB_C)
                t = xrs[r]
                return t[0:64, c, :] if lo64 else t[:, c, :]

            # critical prefix first: the first psum group's j=0 matmuls
            # need only wm[h0][w0:8] + x row 0 (~1 MB)
            W8 = 8
            wms, wts = [], []
            for h in range(ROWS):
                wm = wpool.tile([128, WOUT, 3, COUT], bf16, tag="wm")
                wt = wpool.tile([64, WOUT, 3, COUT], bf16, tag="wt")
                wms.append(wm)
                wts.append(wt)
            # ALL inputs on one ring in exact consumption order: two rings
            # share the 16 SDMA engines round-robin, so splitting inputs
            # reorders arrivals vs need and starves the PE mid-stream.
            # Outputs and shift-copies ride the other ring.
            # wm[h] rides BEFORE the x row tile h+2 (which is only read by
            # that h's j=2 / g=2 matmuls, well after its j=0 start): this
            # keeps every h-boundary starve gap under the ~3.4us HAM MID
            # window so the PE clock never re-throttles mid-stream
            dma_next(wms[0][:, 0:W8], wmain_d[0][:, 0:W8])
            dma_next(wts[0][:, 0:W8], wtail_d[0][:, 0:W8])
            xload(0)
            xload(1)
            xload(2)
            dma_next(wms[0][:, W8:WOUT], wmain_d[0][:, W8:WOUT])
            dma_next(wts[0][:, W8:WOUT], wtail_d[0][:, W8:WOUT])
            for h in range(1, ROWS):
                dma_next(wms[h][:], wmain_d[h])
                xload(h + 2)
                dma_next(wts[h][:], wtail_d[h])

            for h in range(ROWS):
                wm, wt = wms[h], wts[h]
                ot = opool.tile([COUT, WOUT, B], bf16, tag="out")

                for w0 in range(0, WOUT, PW):
                    ps = psum.tile([COUT, PW, B], f32, tag="ps")
                    # all K=128 fulls first (j-major: j=0 only needs x row
                    # h+0, so compute starts before later rows land), then
                    # all K=64 singles: the 128-row <-> 64-row LDWEIGHTS
                    # transition stalls the PE for a pipeline drain
                    # (~180ns), so pay it twice per group, not per location
                    for j in range(3):
                        for dw in range(PW):
                            w = w0 + dw
                            rc = (h + j) * SLAB_C + w
                            nc.tensor.matmul(
                                ps[:, dw, :],
                                wm[:, w, j, :],
                                xap(rc),
                                start=(dw == 0 and j == 0),
                                stop=False,
                            )
                    for dw in range(PW):
                        w = w0 + dw
                        for g in range(3):
                            rc = (h + g) * SLAB_C + (w + 2)
                            nc.tensor.matmul(
                                ps[:, dw, :],
                                wt[:, w, g, :],
                                xap(rc, lo64=True),
                                start=False,
                                stop=(dw == PW - 1 and g == 2),
                            )
                    nc.any.tensor_copy(ot[:, w0 : w0 + PW, :], ps[:])
                    nc.scalar.dma_start(
                        out_d[h][:, w0 : w0 + PW], ot[:, w0 : w0 + PW, :]
                    )

    nc.compile()
    return nc


def get_nc():
    if "nc" not in _nc_cache:
        _nc_cache["nc"] = _build_bass()
    return _nc_cache["nc"]


def _shift(s, d):
    """s: [64, RC, B]; returns s advanced by d blocks along axis 1, zero-filled."""
    out = np.zeros_like(s)
    out[:, : RC - d, :] = s[:, d:, :]
    return out


def pack_inputs(x, weight, bias):
    """Returns list of per-core in_maps (numpy, C-contiguous)."""
    x = np.asarray(x, dtype=np.float32)
    weight = np.asarray(weight, dtype=np.float32)

    # padded x: [B, CIN, H+2, W+2]
    xp = np.zeros((B, CIN, H + 2, W + 2), dtype=np.float32)
    xp[:, :, 1:-1, 1:-1] = x

    # weight -> [h, w, ky, kx, cin, cout]
    wt = np.ascontiguousarray(np.transpose(weight, (2, 3, 4, 5, 0, 1)))

    ky0s = np.array([p[0][0] for p in PAIRS])
    kx0s = np.array([p[0][1] for p in PAIRS])
    ky1s = np.array([p[1][0] for p in PAIRS])
    kx1s = np.array([p[1][1] for p in PAIRS])
    kys = np.array([s[0] for s in SINGLES])
    kxs = np.array([s[1] for s in SINGLES])

    in_maps = []
    for c in range(NCORES):
        h0 = c * ROWS
        # x slab rows h0-1 .. h0+ROWS (SLAB_R rows of padded x)
        slab = xp[:, :, h0 : h0 + SLAB_R, :]  # [B, CIN, SLAB_R, SLAB_C]
        s = np.transpose(slab, (1, 2, 3, 0)).reshape(CIN, RC, B)  # [cin, rc, b]

        wh = wt[h0 : h0 + ROWS]  # [ROWS, w, ky, kx, cin, cout]
        top = wh[:, :, ky0s, kx0s]  # [ROWS, w, j, cin, cout]
        bot = wh[:, :, ky1s, kx1s]
        # -> [ROWS, cin, w, j, cout]
        top = np.transpose(top, (0, 3, 1, 2, 4))
        bot = np.transpose(bot, (0, 3, 1, 2, 4))
        wmain = np.concatenate([top, bot], axis=1)  # [ROWS, 128, w, 3, cout]
        wtail = np.transpose(wh[:, :, kys, kxs], (0, 3, 1, 2, 4))  # [ROWS, cin, w, 3, cout]

        xab = np.concatenate([s, _shift(s, 1)], axis=0).astype(BF16)  # [128, RC, B]
        m = {
            "wmain": np.ascontiguousarray(wmain.astype(BF16)),
            "wtail": np.ascontiguousarray(wtail.astype(BF16)),
        }
        for r in range(SLAB_R):
            m[f"xr{r}"] = np.ascontiguousarray(
                xab[:, r * SLAB_C : (r + 1) * SLAB_C]
            )
        in_maps.append(m)
    return in_maps


def unpack_outputs(results, bias):
    """results: per-core out_maps with 'out' [ROWS, COUT, WOUT, B] bf16."""
    full = np.concatenate(
        [np.asarray(r["out"]).astype(np.float32) for r in results], axis=0
    )
    # [HOUT, COUT, WOUT, B] -> [B, COUT, HOUT, WOUT]
    y = np.ascontiguousarray(np.transpose(full, (3, 1, 0, 2)))
    y += np.asarray(bias, dtype=np.float32)[None]
    return y


def run(in_maps, **kwargs):
    from concourse import bass_utils

    nc = get_nc()
    return bass_utils.run_bass_kernel_spmd(
        nc, in_maps, core_ids=list(range(NCORES)), **kwargs
    )


def kernel(x, weight, bias):
    in_maps = pack_inputs(x, weight, bias)
    res = run(in_maps)
    return unpack_outputs(res.results, bias)


if __name__ == "__main__":
    rng = np.random.default_rng(0)
    x = rng.standard_normal((B, CIN, H, W), dtype=np.float32)
    weight = rng.standard_normal((CIN, COUT, HOUT, WOUT, K, K), dtype=np.float32)
    bias = rng.standard_normal((COUT, HOUT, WOUT), dtype=np.float32)
    y = kernel(x, weight, bias)
    print("out", y.shape, y.dtype)


# revision 32
# speedup vs baseline: 1.0779x; 1.0029x over previous
"""LocallyConnected2d Trainium2 kernel (bf16).

y[b,o,h,w] = sum_{i,ky,kx} x[b,i,h+ky-1,w+kx-1] * weight[i,o,h,w,ky,kx] + bias[o,h,w]

Shapes: x [64,64,32,32], weight [64,64,32,32,3,3], bias [64,32,32] -> y [64,64,32,32].

Strategy
--------
Spatial sharding over H_out: 8 cores x 4 output rows each. All matmul inputs
are bf16 (tolerance is 2e-2; bf16 keeps rel err ~3e-3). Output is written
bf16; bias is added on host during unpack (exact fp32 add, no device cost).

Per output location (h,w): K=576 x M=64(cout) x N=64(batch), split into
6 PSUM-accumulating matmuls:
  j=0,1,2  K=128 chunks pairing offsets (ky=j,kx=0)+(ky=j,kx=1); the bottom
           64 partitions read the x slab shifted by 1 column (host-packed
           second copy), so one rhs AP serves both offsets.
  g=0,1,2  singles (0,2),(1,2),(2,2): K=64 matmuls on partitions 0-63
           reading the unshifted slab at rc3 + 34*g.

Schedule (what the ~2.5x over the fp32 baseline came from, per perfetto):
- Eight consecutive locations share ONE psum bank as a single accumulation
  group (start=True marks the whole 2KB bank pending-zero; later matmuls
  zero-then-accumulate their own slots), so the PSUM drain is one
  512-element copy per 8 locations and the PE never stalls on banks.
- Within a group all 24 K=128 fulls run before the 24 K=64 singles: a
  128-row <-> 64-row LDWEIGHTS transition costs a ~180ns pipeline drain,
  so it is paid twice per group instead of twice per location.
- ALL inputs ride one HWDGE ring in exact consumption order (two rings
  share the 16 SDMA engines round-robin, so splitting inputs reorders
  arrivals vs need and starves the PE); outputs ride the other ring.
  x is split into per-row tiles interleaved with per-h weight chunks.
- A short junk-matmul burst bridges the initial DMA fill so the HAM
  clock-gate is warm (2.4 GHz, ~32ns/MM vs 53 cold) when real work starts.
The kernel is DMA-bound end to end: ~12.8 MB of input at the ~270 GB/s
per-core share of HBM (8 cores active) sets the ~55us floor the matmul
stream trails by one weight chunk.
"""

import sys

sys.path.insert(0, "/opt/trn_rl_repo")

import numpy as np
import ml_dtypes

BF16 = ml_dtypes.bfloat16

B, CIN, COUT, H, W = 64, 64, 64, 32, 32
K = 3
HOUT, WOUT = 32, 32
NCORES = 8
ROWS = HOUT // NCORES  # output rows per core
SLAB_R = ROWS + 2      # x rows needed per core (halo)
SLAB_C = W + 2         # padded width
RC = SLAB_R * SLAB_C   # flattened (row, col) length
RC_LO = 3 * SLAB_C     # x rows 0-2 in the lo tile, 3-5 in the hi tile
PW = 8                 # output locations per psum bank / drain group

# full chunk pairing: j=0..2 -> (j,0)+(j,1); singles (0,2),(1,2),(2,2)
PAIRS = [((0, 0), (0, 1)), ((1, 0), (1, 1)), ((2, 0), (2, 1))]
SINGLES = [(0, 2), (1, 2), (2, 2)]

N_WARM = 18        # junk matmuls to warm the PE clock gate during DMA fill
WARM_N = 512       # their moving free dim

_nc_cache = {}


def _build_bass():
    import concourse.bass as bass
    import concourse.tile as tile
    from concourse import bacc, mybir

    f32 = mybir.dt.float32
    bf16 = mybir.dt.bfloat16
    nc = bacc.Bacc(None, target_bir_lowering=False)

    xr_d = [
        nc.dram_tensor(f"xr{r}", (128, SLAB_C, B), bf16, kind="ExternalInput")
        for r in range(SLAB_R)
    ]
    wmain_d = nc.dram_tensor("wmain", (ROWS, 128, WOUT, 3, COUT), bf16, kind="ExternalInput")
    wtail_d = nc.dram_tensor("wtail", (ROWS, 64, WOUT, 3, COUT), bf16, kind="ExternalInput")
    out_d = nc.dram_tensor("out", (ROWS, COUT, WOUT, B), bf16, kind="ExternalOutput")

    with tile.TileContext(nc) as tc:
        with (
            tc.tile_pool(name="xpool", bufs=1) as xpool,
            tc.tile_pool(name="wpool", bufs=4) as wpool,
            tc.tile_pool(name="opool", bufs=2) as opool,
            tc.tile_pool(name="spool", bufs=1) as spool,
            tc.tile_pool(name="psum", bufs=7, space=bass.MemorySpace.PSUM) as psum,
            tc.tile_pool(name="psumw", bufs=1, space=bass.MemorySpace.PSUM) as psumw,
        ):
            # PE warm-up: junk matmuls keep the HAM activity monitor busy
            # while the inputs stream in, so the real stream starts (and
            # stays) at 2.4 GHz -- warm pace is ~38ns/MM vs 64 cold.
            scratch = spool.tile([128, WARM_N], bf16, tag="scratch")
            nc.vector.memset(scratch[:], 0)
            wps = psumw.tile([64, WARM_N], f32, tag="warm")
            for i in range(N_WARM):
                nc.tensor.matmul(
                    wps[:], scratch[:, 0:64], scratch[:],
                    start=True, stop=True,
                )

            xrs = []
            for r in range(SLAB_R):
                xr = xpool.tile([128, SLAB_C, B], bf16, tag=f"xr{r}")
                xrs.append(xr)

            def dma_next(dst, src):
                nc.sync.dma_start(dst, src)

            def xload(r):
                dma_next(xrs[r][:], xr_d[r][:])

            def xap(rc, lo64=False):
                r, c = divmod(rc, SLAB_C)
                t = xrs[r]
                return t[0:64, c, :] if lo64 else t[:, c, :]

            # critical prefix first: the first psum group's j=0 matmuls
            # need only wm[h0][w0:8] + x row 0 (~1 MB)
            W8 = 16
            wms, wts = [], []
            for h in range(ROWS):
                wm = wpool.tile([128, WOUT, 3, COUT], bf16, tag="wm")
                wt = wpool.tile([64, WOUT, 3, COUT], bf16, tag="wt")
                wms.append(wm)
                wts.append(wt)
            # ALL inputs on one ring in exact consumption order: two rings
            # share the 16 SDMA engines round-robin, so splitting inputs
            # reorders arrivals vs need and starves the PE mid-stream.
            # Outputs and shift-copies ride the other ring.
            # wm[h] rides BEFORE the x row tile h+2 (which is only read by
            # that h's j=2 / g=2 matmuls, well after its j=0 start): this
            # keeps every h-boundary starve gap under the ~3.4us HAM MID
            # window so the PE clock never re-throttles mid-stream
            dma_next(wms[0][:, 0:W8], wmain_d[0][:, 0:W8])
            dma_next(wts[0][:, 0:W8], wtail_d[0][:, 0:W8])
            xload(0)
            xload(1)
            xload(2)
            dma_next(wms[0][:, W8:WOUT], wmain_d[0][:, W8:WOUT])
            dma_next(wts[0][:, W8:WOUT], wtail_d[0][:, W8:WOUT])
            for h in range(1, ROWS):
                dma_next(wms[h][:], wmain_d[h])
                xload(h + 2)
                dma_next(wts[h][:], wtail_d[h])

            for h in range(ROWS):
                wm, wt = wms[h], wts[h]
                ot = opool.tile([COUT, WOUT, B], bf16, tag="out")

                for w0 in range(0, WOUT, PW):
                    ps = psum.tile([COUT, PW, B], f32, tag="ps")
                    # all K=128 fulls first (j-major: j=0 only needs x row
                    # h+0, so compute starts before later rows land), then
                    # all K=64 singles: the 128-row <-> 64-row LDWEIGHTS
                    # transition stalls the PE for a pipeline drain
                    # (~180ns), so pay it twice per group, not per location
                    for j in range(3):
                        for dw in range(PW):
                            w = w0 + dw
                            rc = (h + j) * SLAB_C + w
                            nc.tensor.matmul(
                                ps[:, dw, :],
                                wm[:, w, j, :],
                                xap(rc),
                                start=(dw == 0 and j == 0),
                                stop=False,
                            )
                    for dw in range(PW):
                        w = w0 + dw
                        for g in range(3):
                            rc = (h + g) * SLAB_C + (w + 2)
                            nc.tensor.matmul(
                                ps[:, dw, :],
                                wt[:, w, g, :],
                                xap(rc, lo64=True),
                                start=False,
                                stop=(dw == PW - 1 and g == 2),
                            )
                    nc.any.tensor_copy(ot[:, w0 : w0 + PW, :], ps[:])
                    # last groups: input ring is drained by now -- put
                    # their outs there so the tail's completion receipts
                    # overlap across both rings
                    oeng = nc.sync if (h == ROWS - 1 and w0 >= 16) else nc.scalar
                    oeng.dma_start(
                        out_d[h][:, w0 : w0 + PW], ot[:, w0 : w0 + PW, :]
                    )

    nc.compile()
    return nc


def get_nc():
    if "nc" not in _nc_cache:
        _nc_cache["nc"] = _build_bass()
    return _nc_cache["nc"]


def _shift(s, d):
    """s: [64, RC, B]; returns s advanced by d blocks along axis 1, zero-filled."""
    out = np.zeros_like(s)
    out[:, : RC - d, :] = s[:, d:, :]
    return out


def pack_inputs(x, weight, bias):
    """Returns list of per-core in_maps (numpy, C-contiguous)."""
    x = np.asarray(x, dtype=np.float32)
    weight = np.asarray(weight, dtype=np.float32)

    # padded x: [B, CIN, H+2, W+2]
    xp = np.zeros((B, CIN, H + 2, W + 2), dtype=np.float32)
    xp[:, :, 1:-1, 1:-1] = x

    # weight -> [h, w, ky, kx, cin, cout]
    wt = np.ascontiguousarray(np.transpose(weight, (2, 3, 4, 5, 0, 1)))

    ky0s = np.array([p[0][0] for p in PAIRS])
    kx0s = np.array([p[0][1] for p in PAIRS])
    ky1s = np.array([p[1][0] for p in PAIRS])
    kx1s = np.array([p[1][1] for p in PAIRS])
    kys = np.array([s[0] for s in SINGLES])
    kxs = np.array([s[1] for s in SINGLES])

    in_maps = []
    for c in range(NCORES):
        h0 = c * ROWS
        # x slab rows h0-1 .. h0+ROWS (SLAB_R rows of padded x)
        slab = xp[:, :, h0 : h0 + SLAB_R, :]  # [B, CIN, SLAB_R, SLAB_C]
        s = np.transpose(slab, (1, 2, 3, 0)).reshape(CIN, RC, B)  # [cin, rc, b]

        wh = wt[h0 : h0 + ROWS]  # [ROWS, w, ky, kx, cin, cout]
        top = wh[:, :, ky0s, kx0s]  # [ROWS, w, j, cin, cout]
        bot = wh[:, :, ky1s, kx1s]
        # -> [ROWS, cin, w, j, cout]
        top = np.transpose(top, (0, 3, 1, 2, 4))
        bot = np.transpose(bot, (0, 3, 1, 2, 4))
        wmain = np.concatenate([top, bot], axis=1)  # [ROWS, 128, w, 3, cout]
        wtail = np.transpose(wh[:, :, kys, kxs], (0, 3, 1, 2, 4))  # [ROWS, cin, w, 3, cout]

        xab = np.concatenate([s, _shift(s, 1)], axis=0).astype(BF16)  # [128, RC, B]
        m = {
            "wmain": np.ascontiguousarray(wmain.astype(BF16)),
            "wtail": np.ascontiguousarray(wtail.astype(BF16)),
        }
        for r in range(SLAB_R):
            m[f"xr{r}"] = np.ascontiguousarray(
                xab[:, r * SLAB_C : (r + 1) * SLAB_C]
            )
        in_maps.append(m)
    return in_maps


def unpack_outputs(results, bias):
    """results: per-core out_maps with 'out' [ROWS, COUT, WOUT, B] bf16."""
    full = np.concatenate(
        [np.asarray(r["out"]).astype(np.float32) for r in results], axis=0
    )
    # [HOUT, COUT, WOUT, B] -> [B, COUT, HOUT, WOUT]
    y = np.ascontiguousarray(np.transpose(full, (3, 1, 0, 2)))
    y += np.asarray(bias, dtype=np.float32)[None]
    return y


def run(in_maps, **kwargs):
    from concourse import bass_utils

    nc = get_nc()
    return bass_utils.run_bass_kernel_spmd(
        nc, in_maps, core_ids=list(range(NCORES)), **kwargs
    )


def kernel(x, weight, bias):
    in_maps = pack_inputs(x, weight, bias)
    res = run(in_maps)
    return unpack_outputs(res.results, bias)


if __name__ == "__main__":
    rng = np.random.default_rng(0)
    x = rng.standard_normal((B, CIN, H, W), dtype=np.float32)
    weight = rng.standard_normal((CIN, COUT, HOUT, WOUT, K, K), dtype=np.float32)
    bias = rng.standard_normal((COUT, HOUT, WOUT), dtype=np.float32)
    y = kernel(x, weight, bias)
    print("out", y.shape, y.dtype)


# revision 33
# speedup vs baseline: 1.1070x; 1.0270x over previous
"""LocallyConnected2d Trainium2 kernel (bf16).

y[b,o,h,w] = sum_{i,ky,kx} x[b,i,h+ky-1,w+kx-1] * weight[i,o,h,w,ky,kx] + bias[o,h,w]

Shapes: x [64,64,32,32], weight [64,64,32,32,3,3], bias [64,32,32] -> y [64,64,32,32].

Strategy
--------
Spatial sharding over H_out: 8 cores x 4 output rows each. All matmul inputs
are bf16 (tolerance is 2e-2; bf16 keeps rel err ~3e-3). Output is written
bf16; bias is added on host during unpack (exact fp32 add, no device cost).

Per output location (h,w): K=576 x M=64(cout) x N=64(batch), split into
6 PSUM-accumulating matmuls:
  j=0,1,2  K=128 chunks pairing offsets (ky=j,kx=0)+(ky=j,kx=1); the bottom
           64 partitions read the x slab shifted by 1 column (host-packed
           second copy), so one rhs AP serves both offsets.
  g=0,1,2  singles (0,2),(1,2),(2,2): K=64 matmuls on partitions 0-63
           reading the unshifted slab at rc3 + 34*g.

Schedule (what the ~2.5x over the fp32 baseline came from, per perfetto):
- Eight consecutive locations share ONE psum bank as a single accumulation
  group (start=True marks the whole 2KB bank pending-zero; later matmuls
  zero-then-accumulate their own slots), so the PSUM drain is one
  512-element copy per 8 locations and the PE never stalls on banks.
- Within a group all 24 K=128 fulls run before the 24 K=64 singles: a
  128-row <-> 64-row LDWEIGHTS transition costs a ~180ns pipeline drain,
  so it is paid twice per group instead of twice per location.
- ALL inputs ride one HWDGE ring in exact consumption order (two rings
  share the 16 SDMA engines round-robin, so splitting inputs reorders
  arrivals vs need and starves the PE); outputs ride the other ring.
  x is split into per-row tiles interleaved with per-h weight chunks.
- A short junk-matmul burst bridges the initial DMA fill so the HAM
  clock-gate is warm (2.4 GHz, ~32ns/MM vs 53 cold) when real work starts.
The kernel is DMA-bound end to end: ~12.8 MB of input at the ~270 GB/s
per-core share of HBM (8 cores active) sets the ~55us floor the matmul
stream trails by one weight chunk.
"""

import sys

sys.path.insert(0, "/opt/trn_rl_repo")

import numpy as np
import ml_dtypes

BF16 = ml_dtypes.bfloat16

B, CIN, COUT, H, W = 64, 64, 64, 32, 32
K = 3
HOUT, WOUT = 32, 32
NCORES = 8
ROWS = HOUT // NCORES  # output rows per core
SLAB_R = ROWS + 2      # x rows needed per core (halo)
SLAB_C = W + 2         # padded width
RC = SLAB_R * SLAB_C   # flattened (row, col) length
RC_LO = 3 * SLAB_C     # x rows 0-2 in the lo tile, 3-5 in the hi tile
PW = 8                 # output locations per psum bank / drain group

# full chunk pairing: j=0..2 -> (j,0)+(j,1); singles (0,2),(1,2),(2,2)
PAIRS = [((0, 0), (0, 1)), ((1, 0), (1, 1)), ((2, 0), (2, 1))]
SINGLES = [(0, 2), (1, 2), (2, 2)]

N_WARM = 18        # junk matmuls to warm the PE clock gate during DMA fill
WARM_N = 512       # their moving free dim

_nc_cache = {}


def _build_bass():
    import concourse.bass as bass
    import concourse.tile as tile
    from concourse import bacc, mybir

    f32 = mybir.dt.float32
    bf16 = mybir.dt.bfloat16
    nc = bacc.Bacc(None, target_bir_lowering=False)

    xr_d = [
        nc.dram_tensor(f"xr{r}", (128, SLAB_C, B), bf16, kind="ExternalInput")
        for r in range(SLAB_R)
    ]
    wmain_d = nc.dram_tensor("wmain", (ROWS, 128, WOUT, 3, COUT), bf16, kind="ExternalInput")
    wtail_d = nc.dram_tensor("wtail", (ROWS, 64, WOUT, 3, COUT), bf16, kind="ExternalInput")
    out_d = nc.dram_tensor("out", (ROWS, COUT, WOUT, B), bf16, kind="ExternalOutput")

    with tile.TileContext(nc) as tc:
        with (
            tc.tile_pool(name="xpool", bufs=1) as xpool,
            tc.tile_pool(name="wpool", bufs=4) as wpool,
            tc.tile_pool(name="opool", bufs=2) as opool,
            tc.tile_pool(name="spool", bufs=1) as spool,
            tc.tile_pool(name="psum", bufs=7, space=bass.MemorySpace.PSUM) as psum,
            tc.tile_pool(name="psumw", bufs=1, space=bass.MemorySpace.PSUM) as psumw,
        ):
            # PE warm-up: junk matmuls keep the HAM activity monitor busy
            # while the inputs stream in, so the real stream starts (and
            # stays) at 2.4 GHz -- warm pace is ~38ns/MM vs 64 cold.
            scratch = spool.tile([128, WARM_N], bf16, tag="scratch")
            nc.vector.memset(scratch[:], 0)
            wps = psumw.tile([64, WARM_N], f32, tag="warm")
            for i in range(N_WARM):
                nc.tensor.matmul(
                    wps[:], scratch[:, 0:64], scratch[:],
                    start=True, stop=True,
                )

            xrs = []
            for r in range(SLAB_R):
                xr = xpool.tile([128, SLAB_C, B], bf16, tag=f"xr{r}")
                xrs.append(xr)

            def dma_next(dst, src):
                nc.sync.dma_start(dst, src)

            def xload(r):
                dma_next(xrs[r][:], xr_d[r][:])

            def xap(rc, lo64=False):
                r, c = divmod(rc, SLAB_C)
                t = xrs[r]
                return t[0:64, c, :] if lo64 else t[:, c, :]

            # critical prefix first: the first psum group's j=0 matmuls
            # need only wm[h0][w0:8] + x row 0 (~1 MB)
            W8 = 16
            wms, wts = [], []
            for h in range(ROWS):
                wm = wpool.tile([128, WOUT, 3, COUT], bf16, tag="wm")
                wt = wpool.tile([64, WOUT, 3, COUT], bf16, tag="wt")
                wms.append(wm)
                wts.append(wt)
            # ALL inputs on one ring in exact consumption order: two rings
            # share the 16 SDMA engines round-robin, so splitting inputs
            # reorders arrivals vs need and starves the PE mid-stream.
            # Outputs and shift-copies ride the other ring.
            # wm[h] rides BEFORE the x row tile h+2 (which is only read by
            # that h's j=2 / g=2 matmuls, well after its j=0 start): this
            # keeps every h-boundary starve gap under the ~3.4us HAM MID
            # window so the PE clock never re-throttles mid-stream
            dma_next(wms[0][:, 0:W8], wmain_d[0][:, 0:W8])
            xload(0)
            dma_next(wts[0][:, 0:W8], wtail_d[0][:, 0:W8])
            xload(1)
            xload(2)
            dma_next(wms[0][:, W8:WOUT], wmain_d[0][:, W8:WOUT])
            dma_next(wts[0][:, W8:WOUT], wtail_d[0][:, W8:WOUT])
            for h in range(1, ROWS):
                dma_next(wms[h][:], wmain_d[h])
                xload(h + 2)
                dma_next(wts[h][:], wtail_d[h])

            for h in range(ROWS):
                wm, wt = wms[h], wts[h]
                ot = opool.tile([COUT, WOUT, B], bf16, tag="out")

                for w0 in range(0, WOUT, PW):
                    ps = psum.tile([COUT, PW, B], f32, tag="ps")
                    # all K=128 fulls first (j-major: j=0 only needs x row
                    # h+0, so compute starts before later rows land), then
                    # all K=64 singles: the 128-row <-> 64-row LDWEIGHTS
                    # transition stalls the PE for a pipeline drain
                    # (~180ns), so pay it twice per group, not per location
                    for j in range(3):
                        for dw in range(PW):
                            w = w0 + dw
                            rc = (h + j) * SLAB_C + w
                            nc.tensor.matmul(
                                ps[:, dw, :],
                                wm[:, w, j, :],
                                xap(rc),
                                start=(dw == 0 and j == 0),
                                stop=False,
                            )
                    for dw in range(PW):
                        w = w0 + dw
                        for g in range(3):
                            rc = (h + g) * SLAB_C + (w + 2)
                            nc.tensor.matmul(
                                ps[:, dw, :],
                                wt[:, w, g, :],
                                xap(rc, lo64=True),
                                start=False,
                                stop=(dw == PW - 1 and g == 2),
                            )
                    nc.any.tensor_copy(ot[:, w0 : w0 + PW, :], ps[:])
                    # last groups: input ring is drained by now -- put
                    # their outs there so the tail's completion receipts
                    # overlap across both rings
                    oeng = nc.sync if (h == ROWS - 1 and w0 >= 16) else nc.scalar
                    oeng.dma_start(
                        out_d[h][:, w0 : w0 + PW], ot[:, w0 : w0 + PW, :]
                    )

    nc.compile()
    return nc


def get_nc():
    if "nc" not in _nc_cache:
        _nc_cache["nc"] = _build_bass()
    return _nc_cache["nc"]


def _shift(s, d):
    """s: [64, RC, B]; returns s advanced by d blocks along axis 1, zero-filled."""
    out = np.zeros_like(s)
    out[:, : RC - d, :] = s[:, d:, :]
    return out


def pack_inputs(x, weight, bias):
    """Returns list of per-core in_maps (numpy, C-contiguous)."""
    x = np.asarray(x, dtype=np.float32)
    weight = np.asarray(weight, dtype=np.float32)

    # padded x: [B, CIN, H+2, W+2]
    xp = np.zeros((B, CIN, H + 2, W + 2), dtype=np.float32)
    xp[:, :, 1:-1, 1:-1] = x

    # weight -> [h, w, ky, kx, cin, cout]
    wt = np.ascontiguousarray(np.transpose(weight, (2, 3, 4, 5, 0, 1)))

    ky0s = np.array([p[0][0] for p in PAIRS])
    kx0s = np.array([p[0][1] for p in PAIRS])
    ky1s = np.array([p[1][0] for p in PAIRS])
    kx1s = np.array([p[1][1] for p in PAIRS])
    kys = np.array([s[0] for s in SINGLES])
    kxs = np.array([s[1] for s in SINGLES])

    in_maps = []
    for c in range(NCORES):
        h0 = c * ROWS
        # x slab rows h0-1 .. h0+ROWS (SLAB_R rows of padded x)
        slab = xp[:, :, h0 : h0 + SLAB_R, :]  # [B, CIN, SLAB_R, SLAB_C]
        s = np.transpose(slab, (1, 2, 3, 0)).reshape(CIN, RC, B)  # [cin, rc, b]

        wh = wt[h0 : h0 + ROWS]  # [ROWS, w, ky, kx, cin, cout]
        top = wh[:, :, ky0s, kx0s]  # [ROWS, w, j, cin, cout]
        bot = wh[:, :, ky1s, kx1s]
        # -> [ROWS, cin, w, j, cout]
        top = np.transpose(top, (0, 3, 1, 2, 4))
        bot = np.transpose(bot, (0, 3, 1, 2, 4))
        wmain = np.concatenate([top, bot], axis=1)  # [ROWS, 128, w, 3, cout]
        wtail = np.transpose(wh[:, :, kys, kxs], (0, 3, 1, 2, 4))  # [ROWS, cin, w, 3, cout]

        xab = np.concatenate([s, _shift(s, 1)], axis=0).astype(BF16)  # [128, RC, B]
        m = {
            "wmain": np.ascontiguousarray(wmain.astype(BF16)),
            "wtail": np.ascontiguousarray(wtail.astype(BF16)),
        }
        for r in range(SLAB_R):
            m[f"xr{r}"] = np.ascontiguousarray(
                xab[:, r * SLAB_C : (r + 1) * SLAB_C]
            )
        in_maps.append(m)
    return in_maps


def unpack_outputs(results, bias):
    """results: per-core out_maps with 'out' [ROWS, COUT, WOUT, B] bf16."""
    full = np.concatenate(
        [np.asarray(r["out"]).astype(np.float32) for r in results], axis=0
    )
    # [HOUT, COUT, WOUT, B] -> [B, COUT, HOUT, WOUT]
    y = np.ascontiguousarray(np.transpose(full, (3, 1, 0, 2)))
    y += np.asarray(bias, dtype=np.float32)[None]
    return y


def run(in_maps, **kwargs):
    from concourse import bass_utils

    nc = get_nc()
    return bass_utils.run_bass_kernel_spmd(
        nc, in_maps, core_ids=list(range(NCORES)), **kwargs
    )


def kernel(x, weight, bias):
    in_maps = pack_inputs(x, weight, bias)
    res = run(in_maps)
    return unpack_outputs(res.results, bias)


if __name__ == "__main__":
    rng = np.random.default_rng(0)
    x = rng.standard_normal((B, CIN, H, W), dtype=np.float32)
    weight = rng.standard_normal((CIN, COUT, HOUT, WOUT, K, K), dtype=np.float32)
    bias = rng.standard_normal((COUT, HOUT, WOUT), dtype=np.float32)
    y = kernel(x, weight, bias)
    print("out", y.shape, y.dtype)
